# revision 27
# baseline (speedup 1.0000x reference)
"""Distributed Trainium2 (Bass/Tile) kernel for a causal RoPE attention block.

Reference computation (B=2, S=2048, D=1024, H=16, HD=64):
    qkv = (x @ W_in).reshape(B,S,H,3*HD); q,k,v = split(qkv)
    q,k = rope(q,pos), rope(k,pos); q /= sqrt(HD)
    scores = q @ k^T  (causal masked); attn = softmax(scores)
    out = (attn @ v).reshape(B,S,D) @ W_out

Sharding (8 cores): core c owns batch b=c//4 and heads 4*(c%4)..4*(c%4)+3.
QKV projection is column-parallel and attention fully local per head. The
out-projection is ROW-parallel: each core computes the partial
out^T = W_out[its 256 ctx dims, :]^T-contribution for its batch, and a
per-q-block ReduceScatter(add) over each batch's 4-core group both sums
the partials and scatters the output features - no compute ever waits on
a collective, only the final RS tail is exposed.

Performance structure (PE p-state needs >3us of continuous busy to reach
2.4 GHz, so the PE must never starve):
  - qb-outer loop: attention for both head-pairs of q-block qb runs while
    the PE fills bubbles with the NEXT chunk's Q/K projection + RoPE and
    the V projection; out-proj(qb) + its ReduceScatter overlap qb+1.
  - Causal diagonal tiles compute only the valid column range
    [c0, 512) (scores, exp, AV), and mask-multiply only the 128-wide
    triangular boundary with one shared [128,128] triangle tile.
  - RoPE runs in [128,512] chunks (large DVE ops run ~4x below roofline)
    split across Vector and GpSimd.
  - The two heads of a pair occupy PE row-groups 0-63 / 64-127 in the
    K=64 scores matmuls; one sps2 PSUM pair-tile -> one exp chain.
  - softmax denominators come from an appended ones-column on V; the
    reciprocal uses the fast custom-DVE op.

All matmuls run in bf16 with f32 PSUM accumulation; partial out-proj
sums cross the wire in bf16 and the host converts to f32.
"""

import os
import sys
import numpy as np

for _p in ("/opt/trn_rl_repo", "/root/.axon_site/_ro/trn_rl_repo"):
    if _p not in sys.path and os.path.isdir(_p):
        sys.path.append(_p)

import ml_dtypes
from contextlib import ExitStack

import concourse.bass as bass
import concourse.mybir as mybir
import concourse.tile as tile
from concourse import bacc
from concourse.bass_utils import run_bass_kernel_spmd

F32 = mybir.dt.float32
F32R = mybir.dt.float32r
BF16 = mybir.dt.bfloat16
BF = ml_dtypes.bfloat16

B, S, D, H, HD = 2, 2048, 1024, 16, 64
NCORES = 8
HPC = H // 4   # heads per core = 4
CW = HPC * HD  # per-core qkv slice width = 256
KT = 128       # k tile (partition dim of S^T tiles)
QB = 512       # q block (free dim / PSUM bank)
NKT = S // KT  # 16
NQB = S // QB  # 4
NDT = D // 128 # 8 contraction tiles

TRACE = False
SIM = False
FASTRCP = True
LAST = {}

_cache = {}


def _build(schedule, n_partial):
    """schedule[(kt,qb)] in {'full','skip'}, ('diag',c0) or ('part',idx)."""
    nc = bacc.Bacc(
        "TRN2", target_bir_lowering=False, debug=False, num_devices=NCORES
    )

    xT = nc.dram_tensor("xT", [D, S], BF16, kind="ExternalInput")
    wqp = nc.dram_tensor("wqp", [128, 2048], BF16, kind="ExternalInput")
    wkp = nc.dram_tensor("wkp", [128, 2048], BF16, kind="ExternalInput")
    wvp = nc.dram_tensor("wvp", [128, 2048], BF16, kind="ExternalInput")
    wop = nc.dram_tensor("wop", [128, 2048], BF16, kind="ExternalInput")
    tab = {}
    for t in ("cq", "sq", "ck", "sk"):
        tab[t] = nc.dram_tensor(t, [128, S], BF16, kind="ExternalInput")
    tri = nc.dram_tensor("tri", [128, 128], BF16, kind="ExternalInput")
    perm = nc.dram_tensor("perm", [128, 128], BF16, kind="ExternalInput")
    upsh = nc.dram_tensor("upsh", [64, 128], BF16, kind="ExternalInput")
    if n_partial:
        m01 = nc.dram_tensor("m01", [n_partial, KT, QB], BF16, kind="ExternalInput")
    # out[qb] = summed out^T features [256 (this rank's do slice), 512 q]
    out_e = nc.dram_tensor("out", [NQB, 256, QB], BF16, kind="ExternalOutput")

    with tile.TileContext(nc) as tc, ExitStack() as ctx:
        cst = ctx.enter_context(tc.tile_pool(name="cst", bufs=1))
        dram = ctx.enter_context(tc.tile_pool(name="dram", bufs=1, space="DRAM"))
        raw_p = ctx.enter_context(tc.tile_pool(name="raw", bufs=3))
        rtmp_p = ctx.enter_context(tc.tile_pool(name="rtmp", bufs=6))
        e_p = ctx.enter_context(tc.tile_pool(name="e", bufs=6))
        ctx_p = ctx.enter_context(tc.tile_pool(name="ctx", bufs=3))
        ct_p = ctx.enter_context(tc.tile_pool(name="ct", bufs=3))
        rcp_p = ctx.enter_context(tc.tile_pool(name="rcp", bufs=2))
        rb_p = ctx.enter_context(tc.tile_pool(name="rb", bufs=2))
        oc_p = ctx.enter_context(tc.tile_pool(name="oc", bufs=8))
        # PSUM budget (8 banks): sps2 2x[128,1024] = 4, cx 2x[65,512] = 2,
        # pj 2x[128,512] = 2 (shared by qk/v projections and out-proj).
        sps2_p = ctx.enter_context(tc.tile_pool(name="sps2", bufs=2, space="PSUM"))
        cx_p = ctx.enter_context(tc.tile_pool(name="cx", bufs=2, space="PSUM"))
        pj_p = ctx.enter_context(tc.tile_pool(name="pj", bufs=2, space="PSUM"))

        # ---------------- input DMA (critical-prefix first) ----------------
        # qb0 needs: wkp, xT cols 0-511, wvp, ck/sk, wqp, cq/sq, tri.
        wk_t = cst.tile([128, 2048], BF16, tag="wk", name="wk")
        nc.sync.dma_start(wk_t[:], wkp.ap()[:, :])
        xts = []
        for d in range(NDT):
            t = cst.tile([128, S], BF16, tag=f"xT{d}", name=f"xT{d}")
            nc.sync.dma_start(t[:, 0:QB], xT.ap()[d * 128:(d + 1) * 128, 0:QB])
            xts.append(t)
        wq_t = cst.tile([128, 2048], BF16, tag="wq", name="wq")
        nc.sync.dma_start(wq_t[:], wqp.ap()[:, :])
        tabs = {}
        for tn in ("ck", "sk", "cq", "sq"):
            t = cst.tile([128, S], BF16, tag=tn, name=f"tab_{tn}")
            nc.sync.dma_start(t[:, 0:QB], tab[tn].ap()[:, 0:QB])
            tabs[tn] = t
        wv_t = cst.tile([128, 2048], BF16, tag="wv", name="wv")
        nc.sync.dma_start(wv_t[:], wvp.ap()[:, :])
        tri_t = cst.tile([128, 128], BF16, tag="tri", name="tri")
        nc.sync.dma_start(tri_t[:], tri.ap()[:, :])
        perm_t = cst.tile([128, 128], BF16, tag="perm", name="perm")
        nc.sync.dma_start(perm_t[:], perm.ap()[:, :])
        upsh_t = cst.tile([64, 128], BF16, tag="upsh", name="upsh")
        nc.sync.dma_start(upsh_t[:], upsh.ap()[:, :])
        mts = []
        for i in range(n_partial):
            t = cst.tile([KT, QB], BF16, tag=f"m{i}", name=f"m{i}")
            nc.sync.dma_start(t[:], m01.ap()[i])
            mts.append(t)
        wo_t = cst.tile([128, 2048], BF16, tag="wo", name="wo")

        def emit_chunk_loads(c):
            """Just-in-time bulk loads for projection chunk c: keeps the
            sync DMA rings in use order so rope swap DMAs never queue
            behind bulk input descriptors."""
            csl = slice(c * QB, (c + 1) * QB)
            for d in range(NDT):
                nc.sync.dma_start(
                    xts[d][:, csl], xT.ap()[d * 128:(d + 1) * 128, csl]
                )
            for tn in ("ck", "sk", "cq", "sq"):
                nc.sync.dma_start(tabs[tn][:, csl], tab[tn].ap()[:, csl])

        # ---------------- Q/K projection + RoPE, one 512-col chunk --------
        qrot, krot = [], []
        for i in range(2):
            qrot.append(cst.tile([128, S], BF16, tag=f"qr{i}", name=f"qr{i}"))
            krot.append(cst.tile([128, S], BF16, tag=f"kr{i}", name=f"kr{i}"))

        def emit_qkproj_mm(which, et, c):
            """Projection matmuls + PSUM cast + rotate-half swap DMA. The
            cast is emitted immediately so the pj PSUM slot recycles without
            waiting on the rope chain."""
            wt = wq_t if which == "q" else wk_t
            csl = slice(c * QB, (c + 1) * QB)
            ps = pj_p.tile([128, QB], F32, tag="pj", name=f"pj_{which}{et}{c}")
            for d in range(NDT):
                nc.tensor.matmul(
                    ps[:],
                    wt[:, d * 256 + et * 128:d * 256 + (et + 1) * 128],
                    xts[d][:, csl],
                    start=(d == 0), stop=(d == NDT - 1),
                )
            raw = raw_p.tile([128, QB], BF16, tag="raw", name=f"raw{which}{et}{c}")
            nc.scalar.copy(raw[:], ps[:])
            return which, et, c, raw

        def emit_rope(mm):
            which, et, c, raw = mm
            ctab = tabs["cq" if which == "q" else "ck"]
            stab = tabs["sq" if which == "q" else "sk"]
            rot = (qrot if which == "q" else krot)[et]
            csl = slice(c * QB, (c + 1) * QB)
            t1 = rtmp_p.tile([128, QB], BF16, tag="rtmp", name=f"t1{which}{et}{c}")
            u = rtmp_p.tile([128, QB], BF16, tag="rtmp", name=f"u{which}{et}{c}")
            nc.vector.tensor_mul(t1[:], raw[:], ctab[:, csl])
            nc.vector.tensor_mul(u[:], raw[:], stab[:, csl])
            ps2 = pj_p.tile([128, QB], F32, tag="pj", name=f"rps{which}{et}{c}")
            nc.tensor.matmul(ps2[:], perm_t[:], u[:], start=True, stop=True)
            nc.vector.tensor_add(rot[:, csl], t1[:], ps2[:])

        # ---------------- V projection (two st-tiles per psum) ------------
        # vp[sp][:, (st%2)*4 + hl, 0:64] = v for seq tile st, local head hl;
        # column 64 holds the ones that produce the softmax denominators.
        vps = [None] * (NKT // 2)

        def emit_vproj(sp):
            ps = pj_p.tile([128, 8, 64], F32, tag="pj", name=f"vps{sp}")
            for half in range(2):
                st = 2 * sp + half
                for d in range(NDT):
                    nc.tensor.matmul(
                        ps[:, half * 4:(half + 1) * 4, :],
                        xts[d][:, st * 128:(st + 1) * 128],
                        wv_t[:, d * 256:(d + 1) * 256],
                        start=(d == 0), stop=(d == NDT - 1),
                    )
            vp = cst.tile([128, 8, 65], BF16, tag=f"vp{sp}", name=f"vp{sp}")
            nc.vector.memset(vp[:, :, 64:65], 1.0)
            nc.vector.tensor_copy(vp[:, :, 0:64], ps[:])
            vps[sp] = vp

        # ---------------- attention ----------------
        def emit_attention(i, qb):
            """Both heads of pair i for q block qb. Diagonal causal tiles
            compute only columns [c0, 512) and mask just the 128-wide
            triangular boundary. AV runs two kt behind scores so the exp
            chain is hidden. Returns the pair's [128, 512] bf16 context."""
            q0 = qb * QB
            kts = [kt for kt in range(NKT) if schedule[(kt, qb)][0] != "skip"]
            cps = [
                cx_p.tile([65, QB], F32, tag="cx", name=f"cps{i}{qb}{h}")
                for h in range(2)
            ]
            es = []

            def emit_av(idx, is_last):
                kt = kts[idx]
                e, c0 = es[idx]
                for h in range(2):
                    hl = 2 * i + h
                    nc.tensor.matmul(
                        cps[h][:, c0:QB],
                        vps[kt // 2][:, (kt % 2) * 4 + hl, :],
                        e[:, h * QB + c0:(h + 1) * QB],
                        start=(idx == 0), stop=is_last,
                    )

            stag = 2
            for n, kt in enumerate(kts):
                cls = schedule[(kt, qb)]
                c0 = cls[1] if cls[0] == "diag" else 0
                sps = sps2_p.tile(
                    [KT, 2 * QB], F32, tag="sps2", name=f"sps{i}{qb}{kt}"
                )
                for h in range(2):
                    r0 = h * 64
                    nc.tensor.matmul(
                        sps[:, h * QB + c0:(h + 1) * QB],
                        krot[i][r0:r0 + 64, kt * KT:(kt + 1) * KT],
                        qrot[i][r0:r0 + 64, q0 + c0:q0 + QB],
                        start=True, stop=True,
                    )
                e = e_p.tile([KT, 2 * QB], BF16, tag="e", name=f"e{i}{qb}{kt}")
                if c0 == 0:
                    nc.scalar.activation(
                        e[:], sps[:], mybir.ActivationFunctionType.Exp
                    )
                else:
                    for h in range(2):
                        nc.scalar.activation(
                            e[:, h * QB + c0:(h + 1) * QB],
                            sps[:, h * QB + c0:(h + 1) * QB],
                            mybir.ActivationFunctionType.Exp,
                        )
                if cls[0] == "diag":
                    for h in range(2):
                        b0 = h * QB + c0
                        nc.vector.tensor_mul(
                            e[:, b0:b0 + 128], e[:, b0:b0 + 128], tri_t[:]
                        )
                elif cls[0] == "part":
                    for h in range(2):
                        nc.vector.tensor_mul(
                            e[:, h * QB:(h + 1) * QB],
                            e[:, h * QB:(h + 1) * QB], mts[cls[1]][:],
                        )
                es.append((e, c0))
                if n >= stag:
                    emit_av(n - stag, False)
            for idx in range(max(0, len(kts) - stag), len(kts)):
                emit_av(idx, idx == len(kts) - 1)

            # normalize: ctx[d,q] = cps[d,q] / sigma[q] (sigma = row 64).
            # h1 first (it feeds the partition-shift DMA, the longest pole);
            # both heads' sg/rcp/broadcast chains overlap, and the h1 copy
            # is split across two DMA rings.
            cxt = ctx_p.tile([128, QB], BF16, tag=f"ctx{i}", name=f"cxt{i}{qb}")
            sgs, rcps, rbs = [], [], []
            for h in range(2):
                rcp = rcp_p.tile([1, QB], F32, tag="rcp", name=f"rcp{i}{qb}{h}")
                sg = rcp_p.tile([1, QB], F32, tag="sg", name=f"sg{i}{qb}{h}")
                rb = rb_p.tile([64, QB], F32, tag="rb", name=f"rb{i}{qb}{h}")
                sgs.append(sg)
                rcps.append(rcp)
                rbs.append(rb)
            nc.vector.tensor_copy(sgs[1][:], cps[1][64:65, :])
            nc.scalar.copy(sgs[0][:], cps[0][64:65, :])
            nc.vector.reciprocal_approx_fast(rcps[1][:], sgs[1][:])
            nc.vector.reciprocal_approx_fast(rcps[0][:], sgs[0][:])
            for h in (1, 0):
                nc.gpsimd.partition_broadcast(rbs[h][:], rcps[h][:])
            ct = ct_p.tile([64, QB], BF16, tag="ct", name=f"ct{i}{qb}")
            nc.vector.tensor_mul(ct[:], cps[1][0:64, :], rbs[1][:])
            ps3 = pj_p.tile([128, QB], F32, tag="pj", name=f"ctps{i}{qb}")
            nc.tensor.matmul(
                ps3[64:128, :], upsh_t[:, 64:128], ct[:], start=True, stop=True
            )
            nc.vector.tensor_copy(cxt[64:128, :], ps3[64:128, :])
            nc.vector.tensor_mul(cxt[0:64, :], cps[0][0:64, :], rbs[0][:])
            return cxt

        # ---------------- row-parallel out-proj + per-qb ReduceScatter ----
        # qb<3: one RS per qb over [4, 256, QB] (chunk r = do rows 256r..).
        # qb=3 (the tail): two half-RS over even/odd do-tiles so the last
        # collective exposes only half the data phase.
        stgs = [
            dram.tile([4, 256, QB], BF16, tag=f"stg{qb}", name=f"stg{qb}")
            for qb in range(NQB)
        ]
        rsos = [
            dram.tile([256, QB], BF16, tag=f"rso{qb}", name=f"rso{qb}")
            for qb in range(NQB)
        ]

        bar_in = dram.tile([1, 64], BF16, tag="bar_i", name="bar_i")
        bar_out = dram.tile([8, 64], BF16, tag="bar_o", name="bar_o")

        def emit_barrier(dep_tile):
            # pin the barrier mid-stream: its input DMA depends on qb2's
            # context, so the scheduler cannot hoist the trigger early. The
            # CC core then runs it between RS2 and RS3, re-syncing the cores
            # so the final ReduceScatter's peer-arrival wait collapses.
            nc.sync.dma_start(bar_in[:], dep_tile[0:1, 0:64])
            nc.gpsimd.collective_compute(
                "AllGather",
                mybir.AluOpType.bypass,
                replica_groups=[list(range(NCORES))],
                ins=[bar_in.opt()],
                outs=[bar_out.opt()],
            )

        def emit_rs(ins_tile, outs_tile):
            nc.gpsimd.collective_compute(
                "ReduceScatter",
                mybir.AluOpType.add,
                replica_groups=[[0, 1, 2, 3], [4, 5, 6, 7]],
                ins=[ins_tile.opt()],
                outs=[outs_tile[:]],
            )

        def emit_oproj_tile(qb, cxts, ot):
            ops = pj_p.tile([128, QB], F32, tag="pj", name=f"op{qb}{ot}")
            nc.tensor.matmul(
                ops[:], wo_t[:, ot * 128:(ot + 1) * 128], cxts[0][:],
                start=True, stop=False,
            )
            nc.tensor.matmul(
                ops[:], wo_t[:, 1024 + ot * 128:1024 + (ot + 1) * 128],
                cxts[1][:], start=False, stop=True,
            )
            oc = oc_p.tile([128, QB], BF16, tag="oc", name=f"oc{qb}{ot}")
            nc.vector.tensor_copy(oc[:], ops[:])
            dst = stgs[qb][ot // 2][(ot % 2) * 128:(ot % 2 + 1) * 128, :]
            nc.sync.dma_start(dst, oc[:])

        def emit_oproj(qb, cxts):
            for ot in range(NDT):
                emit_oproj_tile(qb, cxts, ot)
            emit_rs(stgs[qb], rsos[qb])

        # ---------------- emission schedule ----------------
        # chunk 0: pair-0's k+q projections and rope complete first so
        # attention can start while pair-1's chunk-0 proj still runs; then
        # qb-outer attention with the next chunk's projections and V tiles
        # emitted between blocks to fill PE bubbles.
        a = emit_qkproj_mm("k", 0, 0)
        b = emit_qkproj_mm("q", 0, 0)
        emit_rope(a)
        emit_rope(b)
        a = emit_qkproj_mm("k", 1, 0)
        b = emit_qkproj_mm("q", 1, 0)
        emit_rope(a)
        emit_rope(b)
        emit_vproj(0)
        emit_vproj(1)
        nc.sync.dma_start(wo_t[:], wop.ap()[:, :])
        cxts = [None, None]
        for qb in range(NQB):
            cxts[0] = emit_attention(0, qb)
            if qb < NQB - 1:
                c = qb + 1
                emit_chunk_loads(c)
                a = emit_qkproj_mm("k", 0, c)
                b = emit_qkproj_mm("k", 1, c)
                emit_rope(a)
                emit_rope(b)
            cxts[1] = emit_attention(1, qb)
            if qb < NQB - 1:
                c = qb + 1
                a = emit_qkproj_mm("q", 0, c)
                b = emit_qkproj_mm("q", 1, c)
                emit_rope(a)
                emit_rope(b)
                emit_vproj(2 * c)
                emit_vproj(2 * c + 1)
            emit_oproj(qb, cxts)
            if qb == 2:
                emit_barrier(cxts[0])
        # final DRAM->DRAM copies AFTER everything else: a copy waiting on
        # its ReduceScatter must not block later DMAs in the sync queue, so
        # force the scheduler to place them at the very end of the program
        # (tile_wait_until sets a scheduling-time floor, not a runtime wait).
        with tc.tile_wait_until(1.0):
            for qb in range(NQB):
                nc.sync.dma_start(out_e.ap()[qb], rsos[qb][:])

    nc.compile()
    return nc


def _classify_mask(mask):
    """Per (kt,qb) tile classification.

    Returns schedule[(kt,qb)] in {('full',), ('skip',), ('diag', c0),
    ('part', idx)} plus packed generic partial tiles (S^T layout)."""
    m2 = np.asarray(mask).reshape(S, S)  # [q, k] bool
    schedule = {}
    partials = []
    for kt in range(NKT):
        for qb in range(NQB):
            sub = m2[qb * QB:(qb + 1) * QB, kt * KT:(kt + 1) * KT]
            if sub.all():
                schedule[(kt, qb)] = ("full",)
            elif not sub.any():
                schedule[(kt, qb)] = ("skip",)
            else:
                c0 = kt * KT - qb * QB
                if 0 <= c0 <= QB - KT:
                    qi = np.arange(QB)[:, None]
                    ki = np.arange(KT)[None, :]
                    causal = qi >= (ki + c0)
                    if (sub == causal).all():
                        schedule[(kt, qb)] = ("diag", c0)
                        continue
                schedule[(kt, qb)] = ("part", len(partials))
                partials.append(np.ascontiguousarray(sub.T).astype(BF))
    m01 = (
        np.stack(partials)
        if partials
        else np.zeros((0, KT, QB), dtype=BF)
    )
    return schedule, m01


def kernel(inputs, segment_positions, mask, W_in, W_out):
    inputs = np.asarray(inputs, dtype=np.float32)
    segment_positions = np.asarray(segment_positions, dtype=np.int32)
    W_in = np.asarray(W_in, dtype=np.float32)
    W_out = np.asarray(W_out, dtype=np.float32)

    schedule, m01 = _classify_mask(mask)
    key = tuple(sorted(schedule.items()))
    if key not in _cache:
        _cache[key] = _build(schedule, m01.shape[0])
    nc = _cache[key]

    # ---- host-side shard prep (layout/dtype only; no math beyond tables) ----
    # W_in column e maps to head e//192, role (e%192)//64 (q/k/v), dim e%64.
    Wr = W_in.reshape(D, H, 3, HD)
    half = HD // 2
    inv_freq = (1.0 / (10000.0 ** (np.arange(half, dtype=np.float32) / half)))
    tri = np.triu(np.ones((128, 128), dtype=np.float32)).astype(BF)
    upsh = np.zeros((64, 128), dtype=np.float32)
    upsh[np.arange(64), np.arange(64) + 64] = 1.0
    upsh = upsh.astype(BF)
    swap_idx = np.arange(128) ^ 32
    perm = np.zeros((128, 128), dtype=np.float32)
    perm[swap_idx, np.arange(128)] = 1.0
    perm = perm.astype(BF)

    in_maps = []
    for c in range(NCORES):
        b, h0 = c // 4, HPC * (c % 4)
        xTc = np.ascontiguousarray(inputs[b].T).astype(BF)
        wq = Wr[:, h0:h0 + HPC, 0, :].reshape(D, CW)
        wk = Wr[:, h0:h0 + HPC, 1, :].reshape(D, CW)
        wv = Wr[:, h0:h0 + HPC, 2, :].reshape(D, CW)
        wqp = np.ascontiguousarray(
            wq.reshape(NDT, 128, 2, 128).transpose(1, 0, 2, 3).reshape(128, 2048)
        ).astype(BF)
        wkp = np.ascontiguousarray(
            wk.reshape(NDT, 128, 2, 128).transpose(1, 0, 2, 3).reshape(128, 2048)
        ).astype(BF)
        wvp = np.ascontiguousarray(
            wv.reshape(NDT, 128, CW).transpose(1, 0, 2).reshape(128, 2048)
        ).astype(BF)
        wo = W_out[h0 * HD:(h0 + HPC) * HD, :]  # [256, 1024]
        wop = np.ascontiguousarray(
            wo.reshape(2, 128, D).transpose(1, 0, 2).reshape(128, 2048)
        ).astype(BF)

        ang = segment_positions[b].astype(np.float32)[None, :] * inv_freq[:, None]
        c_, s_ = np.cos(ang), np.sin(ang)  # [32, S]
        C64 = np.vstack([c_, c_])
        S64 = np.vstack([-s_, s_])
        C128 = np.vstack([C64, C64]).astype(np.float32)
        S128 = np.vstack([S64, S64]).astype(np.float32)
        scale = 1.0 / np.sqrt(HD).astype(np.float32)
        S128q = (S128 * scale)[swap_idx, :]
        S128k = S128[swap_idx, :]
        im = {
            "xT": xTc, "wqp": wqp, "wkp": wkp, "wvp": wvp, "wop": wop,
            "cq": (C128 * scale).astype(BF), "sq": S128q.astype(BF),
            "ck": C128.astype(BF), "sk": S128k.astype(BF), "tri": tri,
            "perm": perm, "upsh": upsh,
        }
        if m01.shape[0]:
            im["m01"] = m01
        in_maps.append(im)

    if SIM:
        from concourse import bass_interp

        sim = bass_interp.MultiCoreSim(nc, NCORES)
        for c in range(NCORES):
            for k, v in in_maps[c].items():
                sim.cores[c].tensor(k)[:] = v
        sim.simulate(check_with_hw=False)
        results = [
            {"out": np.asarray(sim.cores[c].mem_tensor("out"))}
            for c in range(NCORES)
        ]
        LAST["exec_time_ns"] = None
    else:
        res = run_bass_kernel_spmd(
            nc, in_maps, core_ids=list(range(NCORES)), trace=TRACE
        )
        LAST["exec_time_ns"] = res.exec_time_ns
        LAST["results"] = res
        results = res.results

    # core c (batch b=c//4, rank r=c%4) returns out^T feature rows
    # [256r, 256r+256) for each 512-col q block of its batch.
    out = np.empty((B, S, D), dtype=np.float32)
    for c in range(NCORES):
        r_out = (
            np.asarray(results[c]["out"])
            .astype(np.float32)
            .reshape(NQB, 256, QB)
        )
        b, rk = c // 4, c % 4
        for qb in range(NQB):
            out[b, qb * QB:(qb + 1) * QB, rk * 256:(rk + 1) * 256] = r_out[qb].T
    return out
